# revision 36
# baseline (speedup 1.0000x reference)
"""GQA attention kernel for 8 trn2 NeuronCores (tensor-parallel over heads).

Problem: B=1, S=2048, D=2048, NQ=32 q heads, NKV=8 kv heads, HD=64.
Core i handles q heads 4i..4i+3 and kv head i; out = sum of per-core partials.

v4 (on top of v3): score matmuls row-group packed -- each j-step's two
K=64 score MMs go to PE row groups 0 and 64 (via a partition-swapped copy
QTh2 of the roped Q pairs), so they run concurrently in the array and the
score stream takes half the cycles.  Causal wedge trimmed at 128-column
granularity on the diagonal 512-blocks (scores/exp/PV only cover q >= the
block's diagonal start; one strided [P,2,128] tri-mask per diagonal step
replaces the [P,1024] mask tables).  q-chunk order (2,3,1,0) ends on the
sparsest chunk so the tail epilogue chain is short.  Chunk-3 projections +
rstd Sqrt pulled back into phase 1 (no ACT-table thrash mid-attention);
chunk-3's V + rope stay deferred as first-region PE filler.  Normalize
broadcast via gpsimd partition_broadcast for early chunks, PE ones-matmul
for the latency-critical late chunks.  Out-projection staged into
[P,1024] tiles (2 DMAs per row block instead of 4) with the psum rotating
across opp+bp banks (and across both halves of the borrowed score tiles
on the final chunk) so the drain pipelines instead of serializing on one
bank.  ~40 warmup matmuls pre-warm the HAM clock gate while input DMAs
land; wq DMA'd in halves so the first projection chain starts earlier.

Layout (all seq-transposed; zero device transposes):
  xT  [128, 4(sc), 16(kc), 512] fp16 from host
  Q^T [128 = 2 heads x 64, S] per head pair,  K^T [128 = kv head x2, S]
  V   [S, 64+1] fp16 with ones column (softmax sums fall out of PV matmul)
  S^T pair = K^T_slices.T @ Q^T (2 blocks) -> one exp -> PV: V_ext.T @ expS^T
  row 64 of PV psum = denominators; rec16 = 4096 * approx(1/d) fp16 with the
  2^-12 folded into the broadcast-matmul ones vector (fp16 range safety)
  out-proj: lhsT = O^T directly, partial written to DRAM as fp16
"""

import os
import sys

sys.path.insert(0, "/opt/trn_rl_repo")

import numpy as np

S = 2048
D = 2048
HD = 64
NQ = 32
NKV = 8
P = 128
EPS = 1e-6
SCALE = 0.125  # 1/sqrt(HD)
N_CORES = 8
OUT_DESCALE = 2.0 ** -4  # device writes 16*out (fp16 range management)

_CACHE = {}
LAST_RESULTS = None


def _build_nc():
    import concourse.bass as bass
    import concourse.tile as tile
    from concourse import bacc, mybir

    f16 = mybir.dt.float16
    f32 = mybir.dt.float32
    nc = bacc.Bacc("TRN2", target_bir_lowering=False, debug=False)

    def dram_in(name, shape, dt=f16):
        return nc.dram_tensor(name, list(shape), dt, kind="ExternalInput").ap()

    io = {
        "xt": dram_in("xt", (P, 4, 16, 512)),
        "wq": dram_in("wq", (P, 16, 256)),
        "wk2": dram_in("wk2", (P, 16, P)),
        "wv": dram_in("wv", (P, 16, HD)),
        "wo": dram_in("wo", (P, 2, D)),
        "cos4": dram_in("cos4", (P, S)),
        "sin4s": dram_in("sin4s", (P, S)),
        "rot2": dram_in("rot2", (P, P)),
        "sel2": dram_in("sel2", (2, P)),
        "ones2": dram_in("ones2", (P, 2)),
        "onesd": dram_in("onesd", (1, HD)),
        "ident64": dram_in("ident64", (HD, HD)),
        "trim2": dram_in("trim2", (P, 2, P)),
        "gq2": dram_in("gq2", (P, 1), f32),
        "gk2": dram_in("gk2", (P, 1), f32),
        "out": nc.dram_tensor("out", [S, D], f16, kind="ExternalOutput").ap(),
    }

    from contextlib import ExitStack

    with tile.TileContext(nc) as tc, ExitStack() as ctx:
        _emit(ctx, tc, io, bass, mybir)
    nc.compile()
    return nc


def _emit(ctx, tc, io, bass, mybir):
    nc = tc.nc
    f16 = mybir.dt.float16
    f32 = mybir.dt.float32
    Exp = mybir.ActivationFunctionType.Exp
    Sqrt = mybir.ActivationFunctionType.Sqrt
    mult = mybir.AluOpType.mult

    cpool = ctx.enter_context(tc.tile_pool(name="consts", bufs=1))
    pers = ctx.enter_context(tc.tile_pool(name="persist", bufs=1))

    def cload(name, shape, dt=f16, n_dma=1, eng=None, declare_only=False):
        t = cpool.tile(list(shape), dt, tag=name, name=name)
        if declare_only:
            return t
        e = eng or nc.sync
        if n_dma == 1:
            e.dma_start(t[:], io[name][:])
        else:
            for c in range(n_dma):
                e.dma_start(t[:, c], io[name][:, c])
        return t

    # HAM warmup: dense dependency-free matmuls on a zeroed tile keep the PE
    # activity monitor busy while the input DMAs land, so the first real
    # projection chain runs at the warm 2.4GHz clock instead of 1.2.
    warm = cpool.tile([P, P], f16, tag="warm", name="warm")
    nc.vector.memset(warm[:], 0.0)

    # phase-1 weights on the scalar DMA queue, xt on sync: the two streams
    # overlap.  wq in halves so the first 8-kc accumulation run starts after
    # 0.5MB; xt chunk 0 in quarters for the same reason.  Small constants
    # jump ahead of the bulky xt chunks 1-3 / masks / wo on the sync queue.
    wq = cload("wq", (P, 16, 256), eng=nc.scalar, declare_only=True)
    nc.scalar.dma_start(wq[:, 0:8], io["wq"][:, 0:8])
    nc.scalar.dma_start(wq[:, 8:16], io["wq"][:, 8:16])
    wk2 = cload("wk2", (P, 16, P), eng=nc.scalar)
    wv = cload("wv", (P, 16, HD), eng=nc.scalar)
    xt = cload("xt", (P, 4, 16, 512), declare_only=True)
    for q in range(4):
        nc.sync.dma_start(xt[:, 0, 4 * q : 4 * q + 4], io["xt"][:, 0, 4 * q : 4 * q + 4])
    sel2 = cload("sel2", (2, P))
    ones2 = cload("ones2", (P, 2))
    gq2 = cload("gq2", (P, 1), f32)
    gk2 = cload("gk2", (P, 1), f32)
    ident64 = cload("ident64", (HD, HD))
    rot2 = cload("rot2", (P, P))
    cos4 = cload("cos4", (P, S))
    sin4s = cload("sin4s", (P, S))
    for c in range(1, 4):
        nc.sync.dma_start(xt[:, c], io["xt"][:, c])
    onesd = cload("onesd", (1, HD))
    trim2 = cload("trim2", (P, 2, P))
    wo = cload("wo", (P, 2, D))

    # ---- persistent activations ----
    QTh = pers.tile([P, 2, S], f16, tag="qth")  # roped Q, head pairs
    QTh2 = pers.tile([P, 2, S], f16, tag="qth2")  # QTh with halves swapped
    KTh = pers.tile([P, S], f16, tag="kth")  # rows 64-127 duplicate 0-63
    V = pers.tile([P, 16, HD + 1], f16, tag="v")  # [seq128, kblock, hd+ones]
    OT = pers.tile([P, 2, S], f16, tag="ot")  # attn out transposed
    rstd = pers.tile([2, 3, S], f16, tag="rstd")  # 1/rms per (head, seq)

    # Power-of-2 range management so every fp16 intermediate stays normal:
    # ones col 2^-12 -> rec16 = 4096/d; onesd 2^-2 -> bcs = 1024/d;
    # OT = 1024*O; Wo host-scaled 2^-6 -> device out = 16*true out;
    # the host applies the final 2^-4.
    nc.vector.memset(V[:, :, HD : HD + 1], 2.0 ** -12)

    # SBUF pools shared by phase 1 and the deferred sc=3 work in phase 3
    rawp = ctx.enter_context(tc.tile_pool(name="raw", bufs=4))
    sqp = ctx.enter_context(tc.tile_pool(name="sq", bufs=2))
    vtsp = ctx.enter_context(tc.tile_pool(name="vts", bufs=2))
    nhp = ctx.enter_context(tc.tile_pool(name="nh", bufs=2))
    t12p = ctx.enter_context(tc.tile_pool(name="t12", bufs=4))

    def rope1(pi, g, dst, cs, raws, bct, swt, sbeng=None):
        # sbeng: engine for the SBUF-only elementwise ops (gpsimd for the
        # deferred chunk-3 rope, relieving DVE in the attention window)
        se = sbeng or nc.vector
        bc = bct()
        nc.tensor.matmul(
            bc, sel2[:, :], rstd[:, pi, cs], start=True, stop=True)
        nh = nhp.tile([P, 512], f16, tag="nh", name="nh")
        nc.vector.scalar_tensor_tensor(nh, raws[pi], g[:, :], bc, mult, mult)
        sw = swt()
        nc.tensor.matmul(sw, rot2[:, :], nh, start=True, stop=True)
        t1 = t12p.tile([P, 512], f16, tag="t1", name="t1")
        se.tensor_mul(t1, nh, cos4[:, cs])
        t2 = t12p.tile([P, 512], f16, tag="t2", name="t2")
        nc.vector.tensor_mul(t2, sw, sin4s[:, cs])
        se.tensor_add(dst, t1, t2)

    def rope_targets(cs):
        return [(gq2, QTh[:, 0, cs]), (gq2, QTh[:, 1, cs]), (gk2, KTh[:, cs])]

    def qswap(sc):
        # QTh2[:, p, cs] = QTh[:, p, cs] with partition halves swapped, so a
        # head's roped Q exists at BOTH partition ranges and its score MMs
        # can alternate PE row groups 0/64 (concurrent in the array).
        cs = slice(sc * 512, (sc + 1) * 512)
        for pair in range(2):
            nc.sync.dma_start(QTh2[0:HD, pair, cs], QTh[HD:P, pair, cs])
            nc.sync.dma_start(QTh2[HD:P, pair, cs], QTh[0:HD, pair, cs])

    # ============ Phase 1+2: projections + RMSNorm + RoPE, per seq chunk ====
    # Software-pipelined: the rope for chunk sc-1 is emitted inside chunk sc's
    # projection stream, so its rstd dependency chain (DVE copy -> approx ->
    # ACT Sqrt) is long since resolved and the PE never stalls on it.
    # All 4 chunks project (and Sqrt) here so the attention phase only ever
    # touches the Exp+Copy ACT tables; chunk 3's V blocks and rope stay
    # deferred into the attention phase as qc2 PE filler.
    raws3 = [None] * 3
    with (
        tc.tile_pool(name="ppsum", bufs=2, space="PSUM") as pp,
        tc.tile_pool(name="sspsum", bufs=2, space="PSUM") as ssp,
        tc.tile_pool(name="bcpsum", bufs=1, space="PSUM") as bcp,
        tc.tile_pool(name="swpsum", bufs=1, space="PSUM") as swp,
        tc.tile_pool(name="vtpsum", bufs=1, space="PSUM") as vtp,
        tc.tile_pool(name="vrpsum", bufs=1, space="PSUM") as vrp,
    ):
        wps = pp.tile([P, 512], f32, tag="p", name="wps")
        for _ in range(40):
            nc.tensor.matmul(wps[0:P, 0:P], warm[:, 0:P], warm[:, 0:P],
                             start=True, stop=True)

        rope_pend = []

        def rope(sc, raws):
            cs = slice(sc * 512, (sc + 1) * 512)
            for pi, (g, dst) in enumerate(rope_targets(cs)):
                rope1(pi, g, dst, cs, raws,
                      lambda: bcp.tile([P, 512], f32, tag="bc", name="bc"),
                      lambda: swp.tile([P, 512], f32, tag="sw", name="sw"))
            qswap(sc)

        for sc in range(4):
            cs = slice(sc * 512, (sc + 1) * 512)
            xts = xt[:, sc]  # [P, 16, 512]
            raws = []
            sss = []
            for pi in range(3):  # Qa, Qb, K projections
                if pi == 0:
                    wsl = lambda kc: wq[:, kc, 0:128]
                elif pi == 1:
                    wsl = lambda kc: wq[:, kc, 128:256]
                else:
                    wsl = lambda kc: wk2[:, kc, :]
                ps = pp.tile([P, 512], f32, tag="p", name="ps")
                for kc in range(16):
                    nc.tensor.matmul(
                        ps, wsl(kc), xts[:, kc, :],
                        start=(kc == 0), stop=(kc == 15),
                    )
                raw = rawp.tile([P, 512], f16, tag="raw", name="raw", bufs=6)
                nc.vector.tensor_copy(raw, ps)
                sq = sqp.tile([P, 512], f16, tag="sq", name="sq")
                nc.vector.tensor_mul(sq, raw, raw)
                raws.append(raw)
                sss.append(sq)
            for pi in range(3):  # per-head sum of squares + 1/rms
                ssps = ssp.tile([2, 512], f32, tag="ss", name="ssps")
                nc.tensor.matmul(ssps, ones2[:, :], sss[pi], start=True, stop=True)
                # 1/std = sqrt(64 * approx(1/sumsq)); keeps phase 1 on the
                # Sqrt act table only (no Ln/Exp table thrash -> HAM warm).
                # approx_fast is a raw-bits trick: stage PSUM -> SBUF first.
                ssc = rawp.tile([2, 512], f32, tag="ssc", name="ssc")
                nc.vector.tensor_copy(ssc, ssps)
                r32 = rawp.tile([2, 512], f32, tag="r32s", name="r32s")
                nc.vector.reciprocal_approx_fast(out=r32[:, :], in_=ssc[:, :])
                nc.scalar.activation(rstd[:, pi, cs], r32, Sqrt, scale=float(HD))
            # V projection, transposed (M=512 keeps LDWEIGHTS off the
            # critical path), then flipped back by PE transposes.
            # sc=3's V is deferred into the attention phase (same efficient
            # form, bp psum) as PE filler for the filler-less first region.
            if sc == 3:
                raws3 = raws
            else:
                vt = vtp.tile([HD, 512], f32, tag="vt", name="vt")
                for kc in range(16):
                    nc.tensor.matmul(
                        vt, wv[:, kc, :], xts[:, kc, :],
                        start=(kc == 0), stop=(kc == 15),
                    )
                vts = vtsp.tile([HD, 512], f16, tag="vts", name="vts")
                nc.vector.tensor_copy(vts, vt)
            if rope_pend:
                rope(*rope_pend.pop())  # rope for sc-1: deps long resolved
            if sc < 3:
                for ms in range(4):
                    vr = vrp.tile([P, HD], f16, tag="vr", name="vr")
                    nc.tensor.transpose(
                        vr[:], vts[:, ms * P : (ms + 1) * P], ident64[:, :])
                    nc.scalar.copy(V[:, sc * 4 + ms, 0:HD], vr[:])
            rope_pend.append((sc, raws))
        # rope_pend now holds only (3, raws3): chunk 3's rope is deferred
        # into the attention phase (qc2 filler); raws3 stays live in rawp.
        # preload the Exp act table while phase-1 work drains
        dmy = rawp.tile([2, 16], f16, tag="dmy", name="dmy")
        nc.scalar.activation(dmy[:, :], rstd[:, 0, 0:16], Exp)

    # ================= Phase 3: attention + out-projection =================
    with (
        tc.tile_pool(name="exps", bufs=5) as ep,
        tc.tile_pool(name="rcp", bufs=2) as rcp,
        tc.tile_pool(name="stg", bufs=2) as stgp,
        tc.tile_pool(name="ov", bufs=3) as ovp,
        tc.tile_pool(name="spsum", bufs=2, space="PSUM") as sp,
        tc.tile_pool(name="opsum", bufs=2, space="PSUM") as op_,
        tc.tile_pool(name="bpsum", bufs=1, space="PSUM") as bp,
        tc.tile_pool(name="oppsum", bufs=1, space="PSUM") as opp,
    ):
        pendnorm = []  # deferred normalizes: popped promptly (po recycling)
        pendfill = []  # PE filler units (outproj of prior chunk), paced
        pend3 = []  # chunk-3 deferred units (rope + qswap)

        def flush_norms():
            while pendnorm:
                pendnorm.pop(0)()

        def flush_fill(n=1):
            for _ in range(n):
                if pend3:
                    pend3.pop(0)()
                elif pendfill:
                    pendfill.pop(0)()
                else:
                    break

        def flush():
            flush_norms()
            flush_fill(len(pend3) + len(pendfill))

        # Chunk 3's V + rope + qswap, deferred: first-region PE filler
        # (only q-chunk 3 reads them; projections + rstd ran in phase 1).
        cs3 = slice(3 * 512, 4 * 512)
        v3box = {}

        def v3proj():
            vt = bp.tile([HD, 512], f32, tag="b", name="vt3")
            for kc in range(16):
                nc.tensor.matmul(
                    vt, wv[:, kc, :], xt[:, 3, kc, :],
                    start=(kc == 0), stop=(kc == 15))
            vts = vtsp.tile([HD, 512], f16, tag="vts", name="vts")
            nc.vector.tensor_copy(vts, vt)
            v3box["vts"] = vts

        def v3flip():
            vts = v3box.pop("vts")
            for ms in range(4):
                vr = bp.tile([P, HD], f16, tag="b", name="vr3")
                nc.tensor.transpose(
                    vr[0:P, 0:HD], vts[:, ms * P : (ms + 1) * P], ident64[:, :])
                nc.scalar.copy(V[:, 12 + ms, 0:HD], vr[0:P, 0:HD])

        pend3.append(v3proj)
        for pi in range(3):
            def rope3(pi=pi):
                g = gq2 if pi < 2 else gk2
                dst = QTh[:, pi, cs3] if pi < 2 else KTh[:, cs3]
                rope1(pi, g, dst, cs3, raws3,
                      lambda: opp.tile([P, 512], f32, tag="op", name="bc3"),
                      lambda: opp.tile([P, 512], f32, tag="op", name="sw3"),
                      sbeng=nc.gpsimd)

            pend3.append(rope3)
        pend3.append(lambda: qswap(3))
        pend3.append(v3flip)

        # qc order: dense chunks first (chunk-3 filler + prior outproj keep
        # the PE stream dense), ending on the sparsest chunk so the final
        # epilogue chain (normalize -> outproj -> DMA) is short.
        qc_order = (2, 3, 1, 0)
        qc_last = qc_order[-1]
        for qc in qc_order:
            if qc == 3:
                # chunk-3 rope/V/qswap must be emitted before qc3's attention
                while pend3:
                    pend3.pop(0)()
            qs = slice(qc * 512, (qc + 1) * 512)
            npair = 2 * qc + 2  # kb block pairs (kb = 2j, 2j+1)
            nkb = 2 * npair

            def qrange(j, u):
                # causal trim at 128 granularity: block kb=2j+u only
                # attends q >= its own diagonal start.  Off-diagonal
                # blocks keep the full 512 q columns.
                if j == 2 * qc:
                    qoff = 128 * u
                elif j == 2 * qc + 1:
                    qoff = 256 + 128 * u
                else:
                    qoff = 0
                return qoff, 512 - qoff

            def spair(j, pair, poff):
                # kb=2j on PE row group 0, kb=2j+1 on row group 64:
                # K=64 matmuls to distinct row groups run concurrently,
                # halving the score stream's cycle count.
                ps2 = sp.tile([P, 2, 512], f32, tag="s", name="ps2")
                for u in range(2):
                    kb = 2 * j + u
                    qoff, n = qrange(j, u)
                    src = QTh if (poff == 0) == (u == 0) else QTh2
                    nc.tensor.matmul(
                        ps2[:, u, 0:n],
                        KTh[u * HD : u * HD + HD, kb * P : (kb + 1) * P],
                        src[u * HD : u * HD + HD, pair,
                            qc * 512 + qoff : qc * 512 + qoff + n],
                        start=True, stop=True,
                    )
                es2 = ep.tile([P, 2, 512], f16, tag="e", name="es2")
                if j >= 2 * qc:
                    # diagonal step: per-block exps over the trimmed
                    # ranges, then one strided tri-mask op covering the
                    # first 128 q columns of both blocks
                    for u in range(2):
                        _, n = qrange(j, u)
                        nc.scalar.activation(
                            es2[:, u, 0:n], ps2[:, u, 0:n], Exp, scale=SCALE)
                    nc.vector.tensor_mul(
                        es2[:, :, 0:P], es2[:, :, 0:P], trim2[:, :, :])
                elif qc < 2:
                    # sparse chunks: exp in halves so PV of the even kb
                    # can start while the odd half's exp still runs
                    nc.scalar.activation(
                        es2[:, 0, :], ps2[:, 0, :], Exp, scale=SCALE)
                    nc.scalar.activation(
                        es2[:, 1, :], ps2[:, 1, :], Exp, scale=SCALE)
                else:
                    # per-block exps: PV of the even kb starts while the odd
                    # half's exp still runs, and each psum bank frees as
                    # soon as its half is consumed
                    nc.scalar.activation(
                        es2[:, 0, :], ps2[:, 0, :], Exp, scale=SCALE)
                    nc.scalar.activation(
                        es2[:, 1, :], ps2[:, 1, :], Exp, scale=SCALE)
                return es2

            def ppair(j, es2, po):
                for u in range(2):
                    kb = 2 * j + u
                    qoff, n = qrange(j, u)
                    nc.tensor.matmul(
                        po[:, qoff : qoff + n], V[:, kb, :],
                        es2[:, u, 0:n],
                        start=(kb == 0), stop=(kb == nkb - 1),
                    )

            def den_rec(po, last):
                # denominator reciprocal (DVE, deps ready soon); the
                # normalize trails into the next pairset.  V ones column
                # holds 2^-12, so den = d*2^-12 and rec = 4096/d --
                # comfortably inside fp16 normal range.
                den = rcp.tile([1, 512], f32, tag="den", name="den")
                if last:  # tail: ACT is idle, skip the loaded DVE queue
                    nc.scalar.copy(den, po[HD : HD + 1, :])
                else:
                    nc.vector.tensor_copy(den, po[HD : HD + 1, :])
                rec32 = rcp.tile([1, 512], f32, tag="r32", name="rec32")
                nc.vector.reciprocal_approx_fast(out=rec32[:, :], in_=den[:, :])
                # fold the old onesd 2^-2 into the cast: bcs stays 1024/d
                rec16 = rcp.tile([1, 512], f16, tag="r16", name="rec16")
                if last:
                    nc.scalar.mul(rec16[:, :], rec32[:, :], 0.25)
                else:
                    nc.vector.tensor_scalar_mul(rec16[:, :], rec32[:, :], 0.25)
                return rec16

            def mk_normalize(po, pair, poff, rec16):
                def normalize(po=po, pair=pair, poff=poff, rec16=rec16,
                              qs=qs, qc=qc):
                    bcs = stgp.tile([HD, 512], f16, tag="bcs", name="bcs")
                    if qc in (2, 3):
                        # early chunks: gpsimd broadcast keeps the ones-
                        # matmul off PE / the psum drain off DVE; ample
                        # slack before anything reads OT
                        nc.gpsimd.partition_broadcast(
                            bcs[:, :], rec16[:, :], channels=HD)
                    else:
                        # late chunks: the OT -> outproj chain is critical;
                        # PE matmul + DVE drain has much lower latency
                        bcd = bp.tile([HD, 512], f32, tag="b", name="bcd")
                        nc.tensor.matmul(
                            bcd, onesd[:, :], rec16[:, :], start=True, stop=True)
                        nc.vector.tensor_copy(bcs, bcd)
                    if poff == 0:
                        nc.vector.tensor_mul(OT[0:HD, pair, qs], po[0:HD, :], bcs)
                    else:
                        stg = stgp.tile([HD, 512], f16, tag="stg", name="stg")
                        nc.vector.tensor_mul(stg, po[0:HD, :], bcs)
                        nc.sync.dma_start(OT[HD:P, pair, qs], stg[:])
                return normalize

            # odd heads (DMA partition-shift for OT) first, even heads
            # last: shortens the serial chain from last head to out-proj
            for h in (1, 3, 0, 2):
                pair, poff = h // 2, (h % 2) * HD
                po = op_.tile([HD + 1, 512], f32, tag="o", name="po")
                prev = spair(0, pair, poff)
                flush_norms()  # prior normalizes drain promptly
                flush_fill(1)
                for j in range(1, npair):
                    cur = spair(j, pair, poff)
                    ppair(j - 1, prev, po)
                    prev = cur
                    if j == npair // 2:
                        flush_fill(1)
                ppair(npair - 1, prev, po)
                rec16 = den_rec(po, qc == qc_last and h == 2)
                pendnorm.append(mk_normalize(po, pair, poff, rec16))

            # out-projection units for this q chunk, deferred into next qc.
            # One unit per 128-row block: 4 dc sub-steps accumulate into a
            # [P, 2048] staging tile flushed by a single 512KB DMA -- 4 DMA
            # issues per chunk instead of 16 (the per-DMA ~0.7us issue cost
            # on the sync queue was serializing the tail drain).
            for ms in range(4):
                def outproj(ms=ms, qc=qc):
                    sl = slice(qc * 512 + ms * P, qc * 512 + (ms + 1) * P)
                    box = {}
                    for half in range(2):
                        ov = ovp.tile([P, 1024], f16, tag="ov", name="ov")
                        for u in range(2):
                            dc = 2 * half + u
                            if qc == qc_last:
                                # final chunk's drain: borrow the score psum
                                # tiles (idle now), using BOTH halves of each
                                # so the tail rotates through 4 psum slots
                                if u == 0:
                                    box["t"] = sp.tile(
                                        [P, 1024], f32, tag="s", name="pso2")
                                pso = box["t"][:, u * 512 : (u + 1) * 512]
                            elif dc % 2 == 0:
                                pso = opp.tile([P, 512], f32, tag="op", name="pso")
                            else:
                                # alternate with the bp bank: 2-slot rotation
                                # so the MM stream doesn't wait on each copy
                                pso = bp.tile([P, 512], f32, tag="b", name="psob")
                            for kc in range(2):
                                nc.tensor.matmul(
                                    pso, OT[:, kc, sl],
                                    wo[:, kc, dc * 512 : (dc + 1) * 512],
                                    start=(kc == 0), stop=(kc == 1),
                                )
                            # psum drain split across ACT + DVE
                            dsl = slice(u * 512, (u + 1) * 512)
                            if u == 0:
                                nc.scalar.copy(ov[:, dsl], pso[:])
                            else:
                                nc.vector.tensor_copy(ov[:, dsl], pso[:])
                        # tail chunk: alternate DMA queues (ACT's HWDGE is
                        # idle by then) so the drain isn't issue-serialized
                        deng = nc.scalar if (qc == qc_last and half) else nc.sync
                        deng.dma_start(
                            io["out"][sl, half * 1024 : (half + 1) * 1024], ov[:])

                pendfill.append(outproj)
        flush()


def _prep_core_inputs(i, x, cos, sin, g_q, g_k, Wq, Wk, Wv, Wo):
    c0 = i * 4 * HD
    k0 = i * HD
    x2d = x.reshape(S, D)
    # xt[p, sc, kc, j] = x[sc*512+j, kc*128+p]
    xt = np.ascontiguousarray(
        x2d.T.reshape(16, P, 4, 512).transpose(1, 2, 0, 3).astype(np.float16))
    wq = np.ascontiguousarray(
        Wq[:, c0 : c0 + 256].reshape(16, P, 256).transpose(1, 0, 2)
    ).astype(np.float16)
    wkd = np.concatenate([Wk[:, k0 : k0 + HD]] * 2, axis=1)  # dup kv head
    wk2 = np.ascontiguousarray(
        wkd.reshape(16, P, P).transpose(1, 0, 2)).astype(np.float16)
    wv = np.ascontiguousarray(
        Wv[:, k0 : k0 + HD].reshape(16, P, HD).transpose(1, 0, 2)
    ).astype(np.float16)
    wo = np.ascontiguousarray(
        Wo[c0 : c0 + 2 * P, :].reshape(2, P, D).transpose(1, 0, 2) * 2.0 ** -6
    ).astype(np.float16)
    cosT = cos.T.astype(np.float32)  # [32, S]
    sinT = sin.T.astype(np.float32)
    cos4 = np.tile(cosT, (4, 1)).astype(np.float16)
    sin4s = np.concatenate([-sinT, sinT, -sinT, sinT], axis=0).astype(np.float16)
    gq2 = np.tile(g_q, 2)[:, None].astype(np.float32)
    gk2 = np.tile(g_k, 2)[:, None].astype(np.float32)
    tri = np.triu(np.ones((P, P), dtype=np.float16))  # [k within blk, q]
    trim2 = np.ascontiguousarray(np.stack([tri, tri], axis=1))
    ones2 = np.zeros((P, 2), dtype=np.float16)
    ones2[:HD, 0] = 1.0
    ones2[HD:, 1] = 1.0
    sel2 = np.ascontiguousarray(ones2.T)
    r64 = np.roll(np.eye(HD, dtype=np.float16), 32, axis=0)
    rot2 = np.zeros((P, P), dtype=np.float16)
    rot2[:HD, :HD] = r64
    rot2[HD:, HD:] = r64
    return {
        "xt": xt, "wq": wq, "wk2": wk2, "wv": wv, "wo": wo,
        "cos4": np.ascontiguousarray(cos4),
        "sin4s": np.ascontiguousarray(sin4s),
        "gq2": gq2, "gk2": gk2, "trim2": trim2,
        "ones2": ones2, "sel2": sel2,
        "onesd": np.full((1, HD), 1.0, dtype=np.float16),
        "ident64": np.eye(HD, dtype=np.float16),
        "rot2": rot2,
    }


def kernel(x, cos, sin, g_q, g_k, Wq, Wk, Wv, Wo):
    global LAST_RESULTS
    from concourse.bass_utils import run_bass_kernel_spmd

    if "nc" not in _CACHE:
        _CACHE["nc"] = _build_nc()
    nc = _CACHE["nc"]

    args = [np.asarray(a, dtype=np.float32) for a in
            (x, cos, sin, g_q, g_k, Wq, Wk, Wv, Wo)]
    in_maps = [_prep_core_inputs(i, *args) for i in range(N_CORES)]
    trace = bool(os.environ.get("BASS_TRACE"))
    res = run_bass_kernel_spmd(nc, in_maps, list(range(N_CORES)), trace=trace)
    LAST_RESULTS = res
    out = np.zeros((S, D), dtype=np.float32)
    for r in res.results:
        out += r["out"].astype(np.float32)
    out *= OUT_DESCALE  # undo the device-side power-of-2 range scaling
    return out.reshape(1, S, D)



# revision 37
# speedup vs baseline: 1.0342x; 1.0342x over previous
"""GQA attention kernel for 8 trn2 NeuronCores (tensor-parallel over heads).

Problem: B=1, S=2048, D=2048, NQ=32 q heads, NKV=8 kv heads, HD=64.
Core i handles q heads 4i..4i+3 and kv head i; out = sum of per-core partials.

v4 (on top of v3): score matmuls row-group packed -- each j-step's two
K=64 score MMs go to PE row groups 0 and 64 (via a partition-swapped copy
QTh2 of the roped Q pairs), so they run concurrently in the array and the
score stream takes half the cycles.  Causal wedge trimmed at 128-column
granularity on the diagonal 512-blocks (scores/exp/PV only cover q >= the
block's diagonal start; one strided [P,2,128] tri-mask per diagonal step
replaces the [P,1024] mask tables).  q-chunk order (2,3,1,0) ends on the
sparsest chunk so the tail epilogue chain is short.  Chunk-3 projections +
rstd Sqrt pulled back into phase 1 (no ACT-table thrash mid-attention);
chunk-3's V + rope stay deferred as first-region PE filler.  Normalize
broadcast via gpsimd partition_broadcast for early chunks, PE ones-matmul
for the latency-critical late chunks.  Out-projection staged into
[P,1024] tiles (2 DMAs per row block instead of 4) with the psum rotating
across opp+bp banks (and across both halves of the borrowed score tiles
on the final chunk) so the drain pipelines instead of serializing on one
bank.  ~40 warmup matmuls pre-warm the HAM clock gate while input DMAs
land; wq DMA'd in halves so the first projection chain starts earlier.

Layout (all seq-transposed; zero device transposes):
  xT  [128, 4(sc), 16(kc), 512] fp16 from host
  Q^T [128 = 2 heads x 64, S] per head pair,  K^T [128 = kv head x2, S]
  V   [S, 64+1] fp16 with ones column (softmax sums fall out of PV matmul)
  S^T pair = K^T_slices.T @ Q^T (2 blocks) -> one exp -> PV: V_ext.T @ expS^T
  row 64 of PV psum = denominators; rec16 = 4096 * approx(1/d) fp16 with the
  2^-12 folded into the broadcast-matmul ones vector (fp16 range safety)
  out-proj: lhsT = O^T directly, partial written to DRAM as fp16
"""

import os
import sys

sys.path.insert(0, "/opt/trn_rl_repo")

import numpy as np

S = 2048
D = 2048
HD = 64
NQ = 32
NKV = 8
P = 128
EPS = 1e-6
SCALE = 0.125  # 1/sqrt(HD)
N_CORES = 8
OUT_DESCALE = 2.0 ** -4  # device writes 16*out (fp16 range management)

_CACHE = {}
LAST_RESULTS = None


def _build_nc():
    import concourse.bass as bass
    import concourse.tile as tile
    from concourse import bacc, mybir

    f16 = mybir.dt.float16
    f32 = mybir.dt.float32
    nc = bacc.Bacc("TRN2", target_bir_lowering=False, debug=False)

    def dram_in(name, shape, dt=f16):
        return nc.dram_tensor(name, list(shape), dt, kind="ExternalInput").ap()

    io = {
        "xt": dram_in("xt", (P, 4, 16, 512)),
        "wq": dram_in("wq", (P, 16, 256)),
        "wk2": dram_in("wk2", (P, 16, P)),
        "wv": dram_in("wv", (P, 16, HD)),
        "wo": dram_in("wo", (P, 2, D)),
        "cos4": dram_in("cos4", (P, S)),
        "sin4s": dram_in("sin4s", (P, S)),
        "rot2": dram_in("rot2", (P, P)),
        "sel2": dram_in("sel2", (2, P)),
        "ones2": dram_in("ones2", (P, 2)),
        "onesd": dram_in("onesd", (1, HD)),
        "ident64": dram_in("ident64", (HD, HD)),
        "trim2": dram_in("trim2", (P, 2, P)),
        "gq2": dram_in("gq2", (P, 1), f32),
        "gk2": dram_in("gk2", (P, 1), f32),
        "out": nc.dram_tensor("out", [S, D], f16, kind="ExternalOutput").ap(),
    }

    from contextlib import ExitStack

    with tile.TileContext(nc) as tc, ExitStack() as ctx:
        _emit(ctx, tc, io, bass, mybir)
    nc.compile()
    return nc


def _emit(ctx, tc, io, bass, mybir):
    nc = tc.nc
    f16 = mybir.dt.float16
    f32 = mybir.dt.float32
    Exp = mybir.ActivationFunctionType.Exp
    Sqrt = mybir.ActivationFunctionType.Sqrt
    mult = mybir.AluOpType.mult

    cpool = ctx.enter_context(tc.tile_pool(name="consts", bufs=1))
    pers = ctx.enter_context(tc.tile_pool(name="persist", bufs=1))

    def cload(name, shape, dt=f16, n_dma=1, eng=None, declare_only=False):
        t = cpool.tile(list(shape), dt, tag=name, name=name)
        if declare_only:
            return t
        e = eng or nc.sync
        if n_dma == 1:
            e.dma_start(t[:], io[name][:])
        else:
            for c in range(n_dma):
                e.dma_start(t[:, c], io[name][:, c])
        return t

    # HAM warmup: dense dependency-free matmuls on a zeroed tile keep the PE
    # activity monitor busy while the input DMAs land, so the first real
    # projection chain runs at the warm 2.4GHz clock instead of 1.2.
    warm = cpool.tile([P, P], f16, tag="warm", name="warm")
    nc.vector.memset(warm[:], 0.0)

    # phase-1 weights on the scalar DMA queue, xt on sync: the two streams
    # overlap.  wq in halves so the first 8-kc accumulation run starts after
    # 0.5MB; xt chunk 0 in quarters for the same reason.  Small constants
    # jump ahead of the bulky xt chunks 1-3 / masks / wo on the sync queue.
    wq = cload("wq", (P, 16, 256), eng=nc.scalar, declare_only=True)
    nc.scalar.dma_start(wq[:, 0:8], io["wq"][:, 0:8])
    nc.scalar.dma_start(wq[:, 8:16], io["wq"][:, 8:16])
    wk2 = cload("wk2", (P, 16, P), eng=nc.scalar)
    wv = cload("wv", (P, 16, HD), eng=nc.scalar)
    xt = cload("xt", (P, 4, 16, 512), declare_only=True)
    for q in range(4):
        nc.sync.dma_start(xt[:, 0, 4 * q : 4 * q + 4], io["xt"][:, 0, 4 * q : 4 * q + 4])
    sel2 = cload("sel2", (2, P))
    ones2 = cload("ones2", (P, 2))
    gq2 = cload("gq2", (P, 1), f32)
    gk2 = cload("gk2", (P, 1), f32)
    ident64 = cload("ident64", (HD, HD))
    rot2 = cload("rot2", (P, P))
    cos4 = cload("cos4", (P, S))
    sin4s = cload("sin4s", (P, S))
    for c in range(1, 4):
        nc.sync.dma_start(xt[:, c], io["xt"][:, c])
    onesd = cload("onesd", (1, HD))
    trim2 = cload("trim2", (P, 2, P))
    wo = cload("wo", (P, 2, D))

    # ---- persistent activations ----
    QTh = pers.tile([P, 2, S], f16, tag="qth")  # roped Q, head pairs
    QTh2 = pers.tile([P, 2, S], f16, tag="qth2")  # QTh with halves swapped
    KTh = pers.tile([P, S], f16, tag="kth")  # rows 64-127 duplicate 0-63
    V = pers.tile([P, 16, HD + 1], f16, tag="v")  # [seq128, kblock, hd+ones]
    OT = pers.tile([P, 2, S], f16, tag="ot")  # attn out transposed
    rstd = pers.tile([2, 3, S], f16, tag="rstd")  # 1/rms per (head, seq)

    # Power-of-2 range management so every fp16 intermediate stays normal:
    # ones col 2^-12 -> rec16 = 4096/d; onesd 2^-2 -> bcs = 1024/d;
    # OT = 1024*O; Wo host-scaled 2^-6 -> device out = 16*true out;
    # the host applies the final 2^-4.
    nc.vector.memset(V[:, :, HD : HD + 1], 2.0 ** -12)

    # SBUF pools shared by phase 1 and the deferred sc=3 work in phase 3
    rawp = ctx.enter_context(tc.tile_pool(name="raw", bufs=4))
    sqp = ctx.enter_context(tc.tile_pool(name="sq", bufs=2))
    vtsp = ctx.enter_context(tc.tile_pool(name="vts", bufs=2))
    nhp = ctx.enter_context(tc.tile_pool(name="nh", bufs=2))
    t12p = ctx.enter_context(tc.tile_pool(name="t12", bufs=4))

    def rope1(pi, g, dst, cs, raws, bct, swt, sbeng=None):
        # sbeng: engine for the SBUF-only elementwise ops (gpsimd for the
        # deferred chunk-3 rope, relieving DVE in the attention window)
        se = sbeng or nc.vector
        bc = bct()
        nc.tensor.matmul(
            bc, sel2[:, :], rstd[:, pi, cs], start=True, stop=True)
        nh = nhp.tile([P, 512], f16, tag="nh", name="nh")
        nc.vector.scalar_tensor_tensor(nh, raws[pi], g[:, :], bc, mult, mult)
        sw = swt()
        nc.tensor.matmul(sw, rot2[:, :], nh, start=True, stop=True)
        t1 = t12p.tile([P, 512], f16, tag="t1", name="t1")
        se.tensor_mul(t1, nh, cos4[:, cs])
        t2 = t12p.tile([P, 512], f16, tag="t2", name="t2")
        nc.vector.tensor_mul(t2, sw, sin4s[:, cs])
        se.tensor_add(dst, t1, t2)

    def rope_targets(cs):
        return [(gq2, QTh[:, 0, cs]), (gq2, QTh[:, 1, cs]), (gk2, KTh[:, cs])]

    def qswap(sc):
        # QTh2[:, p, cs] = QTh[:, p, cs] with partition halves swapped, so a
        # head's roped Q exists at BOTH partition ranges and its score MMs
        # can alternate PE row groups 0/64 (concurrent in the array).
        cs = slice(sc * 512, (sc + 1) * 512)
        for pair in range(2):
            nc.sync.dma_start(QTh2[0:HD, pair, cs], QTh[HD:P, pair, cs])
            nc.sync.dma_start(QTh2[HD:P, pair, cs], QTh[0:HD, pair, cs])

    # ============ Phase 1+2: projections + RMSNorm + RoPE, per seq chunk ====
    # Software-pipelined: the rope for chunk sc-1 is emitted inside chunk sc's
    # projection stream, so its rstd dependency chain (DVE copy -> approx ->
    # ACT Sqrt) is long since resolved and the PE never stalls on it.
    # All 4 chunks project (and Sqrt) here so the attention phase only ever
    # touches the Exp+Copy ACT tables; chunk 3's V blocks and rope stay
    # deferred into the attention phase as qc2 PE filler.
    raws3 = [None] * 3
    with (
        tc.tile_pool(name="ppsum", bufs=2, space="PSUM") as pp,
        tc.tile_pool(name="sspsum", bufs=2, space="PSUM") as ssp,
        tc.tile_pool(name="bcpsum", bufs=1, space="PSUM") as bcp,
        tc.tile_pool(name="swpsum", bufs=1, space="PSUM") as swp,
        tc.tile_pool(name="vtpsum", bufs=1, space="PSUM") as vtp,
        tc.tile_pool(name="vrpsum", bufs=1, space="PSUM") as vrp,
    ):
        wps = pp.tile([P, 512], f32, tag="p", name="wps")
        for _ in range(40):
            nc.tensor.matmul(wps[0:P, 0:P], warm[:, 0:P], warm[:, 0:P],
                             start=True, stop=True)

        rope_pend = []

        def rope(sc, raws):
            cs = slice(sc * 512, (sc + 1) * 512)
            for pi, (g, dst) in enumerate(rope_targets(cs)):
                rope1(pi, g, dst, cs, raws,
                      lambda: bcp.tile([P, 512], f32, tag="bc", name="bc"),
                      lambda: swp.tile([P, 512], f32, tag="sw", name="sw"))
            qswap(sc)

        for sc in range(4):
            cs = slice(sc * 512, (sc + 1) * 512)
            xts = xt[:, sc]  # [P, 16, 512]
            raws = []
            sss = []
            for pi in range(3):  # Qa, Qb, K projections
                if pi == 0:
                    wsl = lambda kc: wq[:, kc, 0:128]
                elif pi == 1:
                    wsl = lambda kc: wq[:, kc, 128:256]
                else:
                    wsl = lambda kc: wk2[:, kc, :]
                ps = pp.tile([P, 512], f32, tag="p", name="ps")
                for kc in range(16):
                    nc.tensor.matmul(
                        ps, wsl(kc), xts[:, kc, :],
                        start=(kc == 0), stop=(kc == 15),
                    )
                raw = rawp.tile([P, 512], f16, tag="raw", name="raw", bufs=6)
                nc.vector.tensor_copy(raw, ps)
                sq = sqp.tile([P, 512], f16, tag="sq", name="sq")
                nc.vector.tensor_mul(sq, raw, raw)
                raws.append(raw)
                sss.append(sq)
            for pi in range(3):  # per-head sum of squares + 1/rms
                ssps = ssp.tile([2, 512], f32, tag="ss", name="ssps")
                nc.tensor.matmul(ssps, ones2[:, :], sss[pi], start=True, stop=True)
                # 1/std = sqrt(64 * approx(1/sumsq)); keeps phase 1 on the
                # Sqrt act table only (no Ln/Exp table thrash -> HAM warm).
                # approx_fast is a raw-bits trick: stage PSUM -> SBUF first.
                ssc = rawp.tile([2, 512], f32, tag="ssc", name="ssc")
                nc.vector.tensor_copy(ssc, ssps)
                r32 = rawp.tile([2, 512], f32, tag="r32s", name="r32s")
                nc.vector.reciprocal_approx_fast(out=r32[:, :], in_=ssc[:, :])
                nc.scalar.activation(rstd[:, pi, cs], r32, Sqrt, scale=float(HD))
            # V projection, transposed (M=512 keeps LDWEIGHTS off the
            # critical path), then flipped back by PE transposes.
            # sc=3's V is deferred into the attention phase (same efficient
            # form, bp psum) as PE filler for the filler-less first region.
            if sc == 3:
                raws3 = raws
            else:
                vt = vtp.tile([HD, 512], f32, tag="vt", name="vt")
                for kc in range(16):
                    nc.tensor.matmul(
                        vt, wv[:, kc, :], xts[:, kc, :],
                        start=(kc == 0), stop=(kc == 15),
                    )
                vts = vtsp.tile([HD, 512], f16, tag="vts", name="vts")
                nc.vector.tensor_copy(vts, vt)
            if rope_pend:
                rope(*rope_pend.pop())  # rope for sc-1: deps long resolved
            if sc < 3:
                for ms in range(4):
                    vr = vrp.tile([P, HD], f16, tag="vr", name="vr")
                    nc.tensor.transpose(
                        vr[:], vts[:, ms * P : (ms + 1) * P], ident64[:, :])
                    nc.scalar.copy(V[:, sc * 4 + ms, 0:HD], vr[:])
            rope_pend.append((sc, raws))
        # rope_pend now holds only (3, raws3): chunk 3's rope is deferred
        # into the attention phase (qc2 filler); raws3 stays live in rawp.
        # preload the Exp act table while phase-1 work drains
        dmy = rawp.tile([2, 16], f16, tag="dmy", name="dmy")
        nc.scalar.activation(dmy[:, :], rstd[:, 0, 0:16], Exp)

    # ================= Phase 3: attention + out-projection =================
    with (
        tc.tile_pool(name="exps", bufs=5) as ep,
        tc.tile_pool(name="rcp", bufs=2) as rcp,
        tc.tile_pool(name="stg", bufs=2) as stgp,
        tc.tile_pool(name="ov", bufs=3) as ovp,
        tc.tile_pool(name="spsum", bufs=2, space="PSUM") as sp,
        tc.tile_pool(name="opsum", bufs=2, space="PSUM") as op_,
        tc.tile_pool(name="bpsum", bufs=1, space="PSUM") as bp,
        tc.tile_pool(name="oppsum", bufs=1, space="PSUM") as opp,
    ):
        pendnorm = []  # deferred normalizes: popped promptly (po recycling)
        pendfill = []  # PE filler units (outproj of prior chunk), paced
        pend3 = []  # chunk-3 deferred units (rope + qswap)

        def flush_norms():
            while pendnorm:
                pendnorm.pop(0)()

        def flush_fill(n=1):
            for _ in range(n):
                if pend3:
                    pend3.pop(0)()
                elif pendfill:
                    pendfill.pop(0)()
                else:
                    break

        def flush():
            flush_norms()
            flush_fill(len(pend3) + len(pendfill))

        # Chunk 3's V + rope + qswap, deferred: first-region PE filler
        # (only q-chunk 3 reads them; projections + rstd ran in phase 1).
        cs3 = slice(3 * 512, 4 * 512)
        v3box = {}

        def v3proj():
            vt = bp.tile([HD, 512], f32, tag="b", name="vt3")
            for kc in range(16):
                nc.tensor.matmul(
                    vt, wv[:, kc, :], xt[:, 3, kc, :],
                    start=(kc == 0), stop=(kc == 15))
            vts = vtsp.tile([HD, 512], f16, tag="vts", name="vts")
            nc.vector.tensor_copy(vts, vt)
            v3box["vts"] = vts

        def v3flip():
            vts = v3box.pop("vts")
            for ms in range(4):
                vr = bp.tile([P, HD], f16, tag="b", name="vr3")
                nc.tensor.transpose(
                    vr[0:P, 0:HD], vts[:, ms * P : (ms + 1) * P], ident64[:, :])
                nc.scalar.copy(V[:, 12 + ms, 0:HD], vr[0:P, 0:HD])

        pend3.append(v3proj)
        for pi in range(3):
            def rope3(pi=pi):
                g = gq2 if pi < 2 else gk2
                dst = QTh[:, pi, cs3] if pi < 2 else KTh[:, cs3]
                rope1(pi, g, dst, cs3, raws3,
                      lambda: opp.tile([P, 512], f32, tag="op", name="bc3"),
                      lambda: opp.tile([P, 512], f32, tag="op", name="sw3"),
                      sbeng=nc.gpsimd)

            pend3.append(rope3)
        pend3.append(lambda: qswap(3))
        pend3.append(v3flip)

        # qc order: dense chunks first (chunk-3 filler + prior outproj keep
        # the PE stream dense), ending on the sparsest chunk so the final
        # epilogue chain (normalize -> outproj -> DMA) is short.
        qc_order = (2, 3, 1, 0)
        qc_last = qc_order[-1]
        for qc in qc_order:
            if qc == 3:
                # chunk-3 rope/V/qswap must be emitted before qc3's attention
                while pend3:
                    pend3.pop(0)()
            qs = slice(qc * 512, (qc + 1) * 512)
            npair = 2 * qc + 2  # kb block pairs (kb = 2j, 2j+1)
            nkb = 2 * npair

            def qrange(j, u):
                # causal trim at 128 granularity: block kb=2j+u only
                # attends q >= its own diagonal start.  Off-diagonal
                # blocks keep the full 512 q columns.
                if j == 2 * qc:
                    qoff = 128 * u
                elif j == 2 * qc + 1:
                    qoff = 256 + 128 * u
                else:
                    qoff = 0
                return qoff, 512 - qoff

            def spair(j, pair, poff):
                # kb=2j on PE row group 0, kb=2j+1 on row group 64:
                # K=64 matmuls to distinct row groups run concurrently,
                # halving the score stream's cycle count.
                ps2 = sp.tile([P, 2, 512], f32, tag="s", name="ps2")
                for u in range(2):
                    kb = 2 * j + u
                    qoff, n = qrange(j, u)
                    src = QTh if (poff == 0) == (u == 0) else QTh2
                    nc.tensor.matmul(
                        ps2[:, u, 0:n],
                        KTh[u * HD : u * HD + HD, kb * P : (kb + 1) * P],
                        src[u * HD : u * HD + HD, pair,
                            qc * 512 + qoff : qc * 512 + qoff + n],
                        start=True, stop=True,
                    )
                es2 = ep.tile([P, 2, 512], f16, tag="e", name="es2")
                if j >= 2 * qc:
                    # diagonal step: per-block exps over the trimmed
                    # ranges, then one strided tri-mask op covering the
                    # first 128 q columns of both blocks
                    for u in range(2):
                        _, n = qrange(j, u)
                        nc.scalar.activation(
                            es2[:, u, 0:n], ps2[:, u, 0:n], Exp, scale=SCALE)
                    nc.vector.tensor_mul(
                        es2[:, :, 0:P], es2[:, :, 0:P], trim2[:, :, :])
                elif qc < 2:
                    # sparse chunks: exp in halves so PV of the even kb
                    # can start while the odd half's exp still runs
                    nc.scalar.activation(
                        es2[:, 0, :], ps2[:, 0, :], Exp, scale=SCALE)
                    nc.scalar.activation(
                        es2[:, 1, :], ps2[:, 1, :], Exp, scale=SCALE)
                else:
                    nc.scalar.activation(
                        es2[:, :, :], ps2[:, :, :], Exp, scale=SCALE)
                return es2

            def ppair(j, es2, po):
                for u in range(2):
                    kb = 2 * j + u
                    qoff, n = qrange(j, u)
                    nc.tensor.matmul(
                        po[:, qoff : qoff + n], V[:, kb, :],
                        es2[:, u, 0:n],
                        start=(kb == 0), stop=(kb == nkb - 1),
                    )

            def den_rec(po, last):
                # denominator reciprocal (DVE, deps ready soon); the
                # normalize trails into the next pairset.  V ones column
                # holds 2^-12, so den = d*2^-12 and rec = 4096/d --
                # comfortably inside fp16 normal range.
                den = rcp.tile([1, 512], f32, tag="den", name="den")
                if last:  # tail: ACT is idle, skip the loaded DVE queue
                    nc.scalar.copy(den, po[HD : HD + 1, :])
                else:
                    nc.vector.tensor_copy(den, po[HD : HD + 1, :])
                rec32 = rcp.tile([1, 512], f32, tag="r32", name="rec32")
                nc.vector.reciprocal_approx_fast(out=rec32[:, :], in_=den[:, :])
                # fold the old onesd 2^-2 into the cast: bcs stays 1024/d
                rec16 = rcp.tile([1, 512], f16, tag="r16", name="rec16")
                if last:
                    nc.scalar.mul(rec16[:, :], rec32[:, :], 0.25)
                else:
                    nc.vector.tensor_scalar_mul(rec16[:, :], rec32[:, :], 0.25)
                return rec16

            def mk_normalize(po, pair, poff, rec16):
                def normalize(po=po, pair=pair, poff=poff, rec16=rec16,
                              qs=qs, qc=qc):
                    bcs = stgp.tile([HD, 512], f16, tag="bcs", name="bcs")
                    if qc in (2, 3):
                        # early chunks: gpsimd broadcast keeps the ones-
                        # matmul off PE / the psum drain off DVE; ample
                        # slack before anything reads OT
                        nc.gpsimd.partition_broadcast(
                            bcs[:, :], rec16[:, :], channels=HD)
                    else:
                        # late chunks: the OT -> outproj chain is critical;
                        # PE matmul + DVE drain has much lower latency
                        bcd = bp.tile([HD, 512], f32, tag="b", name="bcd")
                        nc.tensor.matmul(
                            bcd, onesd[:, :], rec16[:, :], start=True, stop=True)
                        nc.vector.tensor_copy(bcs, bcd)
                    if poff == 0:
                        nc.vector.tensor_mul(OT[0:HD, pair, qs], po[0:HD, :], bcs)
                    else:
                        stg = stgp.tile([HD, 512], f16, tag="stg", name="stg")
                        nc.vector.tensor_mul(stg, po[0:HD, :], bcs)
                        nc.sync.dma_start(OT[HD:P, pair, qs], stg[:])
                return normalize

            # odd heads (DMA partition-shift for OT) first, even heads
            # last: shortens the serial chain from last head to out-proj
            for h in (1, 3, 0, 2):
                pair, poff = h // 2, (h % 2) * HD
                po = op_.tile([HD + 1, 512], f32, tag="o", name="po")
                prev = spair(0, pair, poff)
                flush_norms()  # prior normalizes drain promptly
                flush_fill(1)
                for j in range(1, npair):
                    cur = spair(j, pair, poff)
                    ppair(j - 1, prev, po)
                    prev = cur
                    if j == npair // 2:
                        flush_fill(1)
                ppair(npair - 1, prev, po)
                rec16 = den_rec(po, qc == qc_last and h == 2)
                pendnorm.append(mk_normalize(po, pair, poff, rec16))

            # out-projection units for this q chunk, deferred into next qc.
            # One unit per 128-row block: 4 dc sub-steps accumulate into a
            # [P, 2048] staging tile flushed by a single 512KB DMA -- 4 DMA
            # issues per chunk instead of 16 (the per-DMA ~0.7us issue cost
            # on the sync queue was serializing the tail drain).
            for ms in range(4):
                def outproj(ms=ms, qc=qc):
                    sl = slice(qc * 512 + ms * P, qc * 512 + (ms + 1) * P)
                    box = {}
                    for half in range(2):
                        ov = ovp.tile([P, 1024], f16, tag="ov", name="ov")
                        for u in range(2):
                            dc = 2 * half + u
                            if qc == qc_last:
                                # final chunk's drain: borrow the score psum
                                # tiles (idle now), using BOTH halves of each
                                # so the tail rotates through 4 psum slots
                                if u == 0:
                                    box["t"] = sp.tile(
                                        [P, 1024], f32, tag="s", name="pso2")
                                pso = box["t"][:, u * 512 : (u + 1) * 512]
                            elif dc % 2 == 0:
                                pso = opp.tile([P, 512], f32, tag="op", name="pso")
                            else:
                                # alternate with the bp bank: 2-slot rotation
                                # so the MM stream doesn't wait on each copy
                                pso = bp.tile([P, 512], f32, tag="b", name="psob")
                            for kc in range(2):
                                nc.tensor.matmul(
                                    pso, OT[:, kc, sl],
                                    wo[:, kc, dc * 512 : (dc + 1) * 512],
                                    start=(kc == 0), stop=(kc == 1),
                                )
                            # psum drain split across ACT + DVE
                            dsl = slice(u * 512, (u + 1) * 512)
                            if u == 0:
                                nc.scalar.copy(ov[:, dsl], pso[:])
                            else:
                                nc.vector.tensor_copy(ov[:, dsl], pso[:])
                        # tail chunk: alternate DMA queues (ACT's HWDGE is
                        # idle by then) so the drain isn't issue-serialized
                        deng = nc.scalar if (qc == qc_last and half) else nc.sync
                        deng.dma_start(
                            io["out"][sl, half * 1024 : (half + 1) * 1024], ov[:])

                pendfill.append(outproj)
        flush()


def _prep_core_inputs(i, x, cos, sin, g_q, g_k, Wq, Wk, Wv, Wo):
    c0 = i * 4 * HD
    k0 = i * HD
    x2d = x.reshape(S, D)
    # xt[p, sc, kc, j] = x[sc*512+j, kc*128+p]
    xt = np.ascontiguousarray(
        x2d.T.reshape(16, P, 4, 512).transpose(1, 2, 0, 3).astype(np.float16))
    wq = np.ascontiguousarray(
        Wq[:, c0 : c0 + 256].reshape(16, P, 256).transpose(1, 0, 2)
    ).astype(np.float16)
    wkd = np.concatenate([Wk[:, k0 : k0 + HD]] * 2, axis=1)  # dup kv head
    wk2 = np.ascontiguousarray(
        wkd.reshape(16, P, P).transpose(1, 0, 2)).astype(np.float16)
    wv = np.ascontiguousarray(
        Wv[:, k0 : k0 + HD].reshape(16, P, HD).transpose(1, 0, 2)
    ).astype(np.float16)
    wo = np.ascontiguousarray(
        Wo[c0 : c0 + 2 * P, :].reshape(2, P, D).transpose(1, 0, 2) * 2.0 ** -6
    ).astype(np.float16)
    cosT = cos.T.astype(np.float32)  # [32, S]
    sinT = sin.T.astype(np.float32)
    cos4 = np.tile(cosT, (4, 1)).astype(np.float16)
    sin4s = np.concatenate([-sinT, sinT, -sinT, sinT], axis=0).astype(np.float16)
    gq2 = np.tile(g_q, 2)[:, None].astype(np.float32)
    gk2 = np.tile(g_k, 2)[:, None].astype(np.float32)
    tri = np.triu(np.ones((P, P), dtype=np.float16))  # [k within blk, q]
    trim2 = np.ascontiguousarray(np.stack([tri, tri], axis=1))
    ones2 = np.zeros((P, 2), dtype=np.float16)
    ones2[:HD, 0] = 1.0
    ones2[HD:, 1] = 1.0
    sel2 = np.ascontiguousarray(ones2.T)
    r64 = np.roll(np.eye(HD, dtype=np.float16), 32, axis=0)
    rot2 = np.zeros((P, P), dtype=np.float16)
    rot2[:HD, :HD] = r64
    rot2[HD:, HD:] = r64
    return {
        "xt": xt, "wq": wq, "wk2": wk2, "wv": wv, "wo": wo,
        "cos4": np.ascontiguousarray(cos4),
        "sin4s": np.ascontiguousarray(sin4s),
        "gq2": gq2, "gk2": gk2, "trim2": trim2,
        "ones2": ones2, "sel2": sel2,
        "onesd": np.full((1, HD), 1.0, dtype=np.float16),
        "ident64": np.eye(HD, dtype=np.float16),
        "rot2": rot2,
    }


def kernel(x, cos, sin, g_q, g_k, Wq, Wk, Wv, Wo):
    global LAST_RESULTS
    from concourse.bass_utils import run_bass_kernel_spmd

    if "nc" not in _CACHE:
        _CACHE["nc"] = _build_nc()
    nc = _CACHE["nc"]

    args = [np.asarray(a, dtype=np.float32) for a in
            (x, cos, sin, g_q, g_k, Wq, Wk, Wv, Wo)]
    in_maps = [_prep_core_inputs(i, *args) for i in range(N_CORES)]
    trace = bool(os.environ.get("BASS_TRACE"))
    res = run_bass_kernel_spmd(nc, in_maps, list(range(N_CORES)), trace=trace)
    LAST_RESULTS = res
    out = np.zeros((S, D), dtype=np.float32)
    for r in res.results:
        out += r["out"].astype(np.float32)
    out *= OUT_DESCALE  # undo the device-side power-of-2 range scaling
    return out.reshape(1, S, D)

